# revision 27
# baseline (speedup 1.0000x reference)
"""Trainium2 8-core kernel for the LSTM seq2seq + attention + vocab-projection model.

Strategy:
  - LSTM recurrence: tensor-parallel over the gate dimension. Core m owns
    h-slice [m*128:(m+1)*128) and computes the 4 gate rows for that slice
    (packed on host in order [f, i, o, g], 128 rows each). After each step the
    h-slices are exchanged so every core holds the full h for the next step.
  - Exchange modes (CC_MODE):
      "rdma": direct SBUF->SBUF remote_dma_broadcast to each peer (XOR-relative
              single destinations). Receiver position k holds the slice of peer
              (own_id ^ k); all weight K-blocks are XOR-permuted per core on the
              host so the math stays consistent. Per-peer semaphores gate each
              K-tile group of the next matmul, so the exchange overlaps the
              following step's matmuls. Zero DRAM bounces, no ncfw collective.
      "cc":   AllGather via ncfw collective with DRAM bounce buffers.
      "sbuf"/"dram": timing-attribution fakes (wrong results).
  - The additive attention collapses: softmax over (dec@wd + enc@we + b) along
    the encoder axis is independent of the decoder position, so the context
    vector is per-batch constant. ctx[b] = softmax_e(enc_out[b]@we) @ enc_out[b].
  - Final projection is vocab-sharded: core m computes rows [m*4000,(m+1)*4000)
    (padded to 4096) of  out = dec_out @ fc_w[:, :H].T + (ctx @ fc_w[:, H:].T
    + fc_b).  The fc work is emitted as background chunks interleaved into the
    decoder phase so it runs in PE gaps while steps wait on the exchange.
  - The x @ Wih.T + b term is precomputed for all steps (xw) and folded into
    the gate PSUM with an identity-weight matmul, so the per-step cell update
    is ACT/DVE only: sigmoid/tanh straight from PSUM.
Token index convention: tau = t*16 + b  (time-major, batch inner).
"""

import os
import sys

for _p in ("/opt/trn_rl_repo", "/root/.axon_site/_ro/trn_rl_repo"):
    if os.path.isdir(_p) and _p not in sys.path:
        sys.path.insert(0, _p)

import numpy as np
import ml_dtypes

import concourse.bass as bass
import concourse.bacc as bacc
import concourse.tile as tile
from concourse.tile import add_dep_helper
from concourse import mybir
from concourse.bass_utils import run_bass_kernel_spmd

BF16 = ml_dtypes.bfloat16
DT = mybir.dt
AF = mybir.ActivationFunctionType
ALU = mybir.AluOpType

B = 16
T = 128          # both encoder and decoder length
H = 1024
V = 32000
NC = 8
HL = H // NC     # 128  h-slice per core
KT = H // 128    # 8    K tiles of the hidden dim
T2 = B * T       # 2048 tokens
VL = V // NC     # 4000 real vocab rows per core
VLP = 4096       # padded vocab rows per core
MT = VLP // 128  # 32   vocab M-tiles per core
# gate order on device: [f, i, o, g]; torch rows are [i, f, g, o]
GATE_SRC = (1, 0, 3, 2)

CC_MODE = "cc"  # "cc" | "rdma" | "dram" | "sbuf"
RD_EXCHANGE = False  # rdma flavor: True = 3-round recursive doubling, False = 7-peer one-shot
CC_DMA = "act"  # engine for exchange bounce DMAs: "sync" | "gpsimd" | "act"
CC_SHARED = False  # Shared addr_space for AllGather outputs
DEBUG_DUMP = False
NO_BG = False
DMA_SCRATCH = 16384
H_BUFS = 3


def _bcast(ap, dim, count):
    """Insert a [step=0, count] broadcast dim at position `dim` of ap.ap."""
    l = [list(d) for d in ap.ap]
    l.insert(dim, [0, count])
    return bass.AP(ap.tensor, ap.offset, l)


def build_nc(n_steps=T, reps=1):
    nc = bacc.Bacc(
        "TRN2", target_bir_lowering=False, debug=False, num_devices=NC,
        dynamic_dma_scratch_size=DMA_SCRATCH,
    )

    # ---- kernel I/O (per-core shards; all pre-laid-out on host) ----
    xet = nc.dram_tensor("xet", [H, T2], DT.bfloat16, kind="ExternalInput")
    xdt = nc.dram_tensor("xdt", [H, T2], DT.bfloat16, kind="ExternalInput")
    whe = nc.dram_tensor("whe", [128, KT * 512], DT.bfloat16, kind="ExternalInput")
    wie = nc.dram_tensor("wie", [128, KT * 512], DT.bfloat16, kind="ExternalInput")
    whd = nc.dram_tensor("whd", [128, KT * 512], DT.bfloat16, kind="ExternalInput")
    wid = nc.dram_tensor("wid", [128, KT * 512], DT.bfloat16, kind="ExternalInput")
    be = nc.dram_tensor("be", [128, 4], DT.float32, kind="ExternalInput")
    bd = nc.dram_tensor("bd", [128, 4], DT.float32, kind="ExternalInput")
    fw1 = nc.dram_tensor("fw1", [128, MT * KT * 128], DT.bfloat16, kind="ExternalInput")
    fw2 = nc.dram_tensor("fw2", [128, MT * KT * 128], DT.bfloat16, kind="ExternalInput")
    fcb = nc.dram_tensor("fcb", [128, MT], DT.float32, kind="ExternalInput")
    wet = nc.dram_tensor("wet", [128, KT], DT.bfloat16, kind="ExternalInput")
    idt = nc.dram_tensor("idt", [128, 128], DT.bfloat16, kind="ExternalInput")
    out = nc.dram_tensor("out", [VLP, T2], DT.float32, kind="ExternalOutput")

    rdma = CC_MODE == "rdma"
    # Recursive-doubling exchange: 3 rounds (peer ^1, ^2, ^4), one sem per
    # round. Round r delivers positions [2^(r-1), 2^r) of the XOR-permuted
    # h layout (position k = slice of peer own^k), +2 per step per sem.
    nsems = 3 if RD_EXCHANGE else KT - 1
    rsems = [nc.alloc_semaphore(f"rsem{k}") for k in range(nsems)] if rdma else None
    lsem = nc.alloc_semaphore("lsem") if rdma else None
    # Register-valued wait thresholds: Tile's single-core scheduling sim
    # cannot model remote sem increments (constant-threshold waits on rsems
    # deadlock it), but it treats register-valued thresholds as satisfiable
    # while the HW wait still enforces the real value at runtime.
    regs = None
    if rdma:
        regs = {
            "pe": nc.alloc_register(mybir.EngineType.PE, "pethr"),
            "dve": nc.alloc_register(mybir.EngineType.DVE, "dvethr"),
            "pool": nc.alloc_register(mybir.EngineType.Pool, "poolthr"),
        }

    with tile.TileContext(nc) as tc:
        with (
            tc.tile_pool(name="persist", bufs=1) as pp,
            tc.tile_pool(name="work", bufs=3) as wk,
            tc.tile_pool(name="wstream", bufs=2) as ws,
            tc.tile_pool(name="work1", bufs=1) as wk1,
            tc.tile_pool(name="hpool", bufs=H_BUFS) as hp,
            tc.tile_pool(name="dcc", bufs=4, space="DRAM") as dcc,
            tc.tile_pool(name="dccs", bufs=4, space="DRAM") as dccs,
            tc.tile_pool(name="paux", bufs=3, space="PSUM") as paux,
            tc.tile_pool(name="pgate", bufs=1, space="PSUM") as pgate,
        ):
            gstep = [0]
            for _rep in range(reps):
                _emit_once(
                    nc, tc, pp, wk, wk1, hp, ws, dcc, dccs, paux, pgate, n_steps, gstep,
                    rsems, lsem, regs, _rep,
                    xet, xdt, whe, wie, whd, wid, be, bd, fw1, fw2, fcb, wet,
                    idt, out,
                )

    nc.compile()
    return nc


def _emit_once(
    nc, tc, pp, wk, wk1, hp, ws, dcc, dccs, paux, pgate, n_steps, gstep, rsems, lsem,
    regs, rep,
    xet, xdt, whe, wie, whd, wid, be, bd, fw1, fw2, fcb, wet, idt, out,
):
    rdma = CC_MODE == "rdma"
    gate_pe = [None]   # latest anchor instruction on PE
    gate_dve = [None]  # latest anchor instruction on DVE

    def _anchor(a, b):
        if b is not None:
            add_dep_helper(a.ins, b.ins, sync=False, reason="stream anchor")

    def anchored_waits(eng, reg, sems, thr, anchor):
        """Reg-threshold waits on all rsems at `thr`, chained into the
        engine's anchor stream (reg_mov first, then one wait per sem)."""
        prev = anchor[0]
        mov = eng.reg_mov(reg, thr)
        _anchor(mov, prev)
        prev = mov
        for s in sems:
            w = eng.wait_ge(s, reg)
            _anchor(w, prev)
            prev = w
        return prev
    # ---- persistent SBUF tensors ----
    whe_sb = pp.tile([128, KT * 512], DT.bfloat16, tag="whe")
    whd_sb = pp.tile([128, KT * 512], DT.bfloat16, tag="whd")
    wie_sb = pp.tile([128, KT * 512], DT.bfloat16, tag="wih")
    wid_sb = pp.tile([128, KT * 512], DT.bfloat16, tag="wih")
    be_sb = pp.tile([128, 4], DT.float32, tag="be")
    bd_sb = pp.tile([128, 4], DT.float32, tag="bd")
    fcb_sb = pp.tile([128, MT], DT.float32, tag="fcb")
    wet_sb = pp.tile([128, KT], DT.bfloat16, tag="wet")
    id_sb = pp.tile([128, 128], DT.bfloat16, tag="ident")
    xt_sb = pp.tile([128, KT * T2], DT.bfloat16, tag="xt")       # 32KB/p
    xwe_sb = pp.tile([128, 4 * T2], DT.bfloat16, tag="xwe")      # 16KB/p
    xwd_sb = pp.tile([128, 4 * T2], DT.bfloat16, tag="xwd")      # 16KB/p
    hist_e = pp.tile([128, n_steps * 128], DT.bfloat16, tag="he")
    hist_d = pp.tile([128, n_steps * 128], DT.bfloat16, tag="hd")
    h0_sb = pp.tile([128, 128], DT.bfloat16, tag="h0")
    c_sb = pp.tile([128, B], DT.float32, tag="c")
    se_sb = pp.tile([1, T2], DT.float32, tag="se")
    seT_sb = pp.tile([B, n_steps], DT.float32, tag="seT")
    attn_sb = pp.tile([B, n_steps], DT.bfloat16, tag="attn")
    abc_sb = pp.tile([128, B * n_steps], DT.bfloat16, tag="abc")
    ctx_sb = pp.tile([128, KT * B], DT.bfloat16, tag="ctx")
    bias2_sb = pp.tile([128, MT * B], DT.float32, tag="bias2")

    nc.sync.dma_start(whe_sb[:], whe[:])
    nc.sync.dma_start(whd_sb[:], whd[:])
    nc.sync.dma_start(wie_sb[:], wie[:])
    nc.sync.dma_start(wid_sb[:], wid[:])
    nc.sync.dma_start(be_sb[:], be[:])
    nc.sync.dma_start(bd_sb[:], bd[:])
    nc.sync.dma_start(fcb_sb[:], fcb[:])
    nc.sync.dma_start(wet_sb[:], wet[:])
    nc.sync.dma_start(id_sb[:], idt[:])
    nc.vector.memset(c_sb[:], 0.0)

    # h0 := zeros, doubling as a cross-core barrier in rdma mode: semaphores
    # are cleared before this core contributes to the AllGather, and every
    # core's step-0 matmuls read h0 (the gather output), so no core can send
    # before every core has cleared.
    zin = dcc.tile([128, B], DT.bfloat16, tag="zin")
    zout = dcc.tile([NC * 128, B], DT.bfloat16, tag="zout")
    zsb = wk.tile([128, B], DT.bfloat16, tag="zsb")
    if rdma and rep == 0:
        with tc.tile_critical():
            for s in rsems:
                nc.gpsimd.sem_clear(s)
            nc.gpsimd.sem_clear(lsem)
            nc.gpsimd.memset(zsb[:], 0.0)
    else:
        nc.gpsimd.memset(zsb[:], 0.0)
    nc.gpsimd.dma_start(zin[:], zsb[:])
    nc.gpsimd.collective_compute(
        "AllGather",
        ALU.bypass,
        replica_groups=[list(range(NC))],
        ins=[zin[:].opt()],
        outs=[zout[:].opt()],
    )
    nc.sync.dma_start(
        h0_sb[:].rearrange("p (k b) -> p k b", k=KT),
        zout[:].rearrange("(k p) b -> p k b", p=128),
    )

    def load_xt(src):
        nc.sync.dma_start(
            xt_sb[:].rearrange("p (k n) -> p k n", k=KT),
            src[:].rearrange("(k p) n -> p k n", p=128),
        )

    def xw_chunk(xw_sb, wih_sb, b_sb, j, nb):
        """One (gate j, 512-token block nb) chunk of xw = x @ Wih_m.T + b."""
        ps = paux.tile([128, 512], DT.float32, tag="aux")
        for k in range(KT):
            nc.tensor.matmul(
                ps[:],
                lhsT=wih_sb[:, k * 512 + j * 128 : k * 512 + (j + 1) * 128],
                rhs=xt_sb[:, k * T2 + nb * 512 : k * T2 + (nb + 1) * 512],
                start=(k == 0),
                stop=(k == KT - 1),
            )
        nc.vector.tensor_scalar_add(
            xw_sb[:, j * T2 + nb * 512 : j * T2 + (nb + 1) * 512],
            ps[:],
            b_sb[:, j : j + 1],
        )

    def lstm_step(hist, hprev_ap, xw_sb, whh_sb, t, g):
        """g = global step counter (1-based) for semaphore thresholds."""
        pg = pgate.tile([128, 2048], DT.float32, tag="g")
        # fold xw into PSUM: pg[j] = I.T @ xw_t[j]  (start=True clears)
        for j in range(4):
            nc.tensor.matmul(
                pg[:, j * 512 : j * 512 + B],
                lhsT=id_sb[:],
                rhs=xw_sb[:, j * T2 + t * B : j * T2 + (t + 1) * B],
                start=True,
                stop=False,
            )

        def kmm2(k, last):
            first = None
            for j in range(4):
                mm = nc.tensor.matmul(
                    pg[:, j * 512 : j * 512 + B],
                    lhsT=whh_sb[:, k * 512 + j * 128 : k * 512 + (j + 1) * 128],
                    rhs=hprev_ap[:, k * B : (k + 1) * B],
                    start=False,
                    stop=last,
                )
                if first is None:
                    first = mm
            return first, mm

        def kmm(k, last):
            return kmm2(k, last)[1]

        if rdma and g > 1:
            prev = kmm(0, False)
            mov = nc.tensor.reg_mov(regs["pe"], 2 * (g - 1))
            _anchor(mov, prev)
            prev = mov
            # position k arrives with this sem (RD: per round; one-shot: per peer)
            ksem = {1: 0, 2: 1, 4: 2} if RD_EXCHANGE else {k: k - 1 for k in range(1, KT)}
            for k in range(1, KT):
                si = ksem.get(k)
                if si is not None:
                    w = nc.tensor.wait_ge(rsems[si], regs["pe"])
                    _anchor(w, prev)
                    first, prev = kmm2(k, k == KT - 1)
                    _anchor(first, w)
                else:
                    prev = kmm(k, k == KT - 1)
            gate_pe[0] = prev
        else:
            for k in range(KT):
                last = kmm(k, k == KT - 1)
            gate_pe[0] = last

        # cell update straight from PSUM
        a = wk.tile([128, 64], DT.float32, tag="act")
        nc.scalar.activation(
            a[:].rearrange("p (j b) -> p j b", j=4)[:, 0:3, :],
            pg[:].rearrange("p (j x) -> p j x", j=4)[:, 0:3, 0:B],
            AF.Sigmoid,
        )
        nc.scalar.activation(a[:, 48:64], pg[:, 3 * 512 : 3 * 512 + B], AF.Tanh)
        m1 = wk.tile([128, B], DT.float32, tag="m1")
        m2 = wk.tile([128, B], DT.float32, tag="m2")
        nc.vector.tensor_mul(m1[:], a[:, 0:16], c_sb[:])
        nc.vector.tensor_mul(m2[:], a[:, 16:32], a[:, 48:64])
        nc.vector.tensor_add(c_sb[:], m1[:], m2[:])
        tct = wk.tile([128, B], DT.float32, tag="tct")
        nc.scalar.activation(tct[:], c_sb[:], AF.Tanh)
        dst = hist[:, t * 128 : (t + 1) * 128]
        if rdma and not RD_EXCHANGE:
            # h lands straight in this step's hist slot (position 0); the
            # sends read it from there, so the slot is never reused within a
            # rep and no send-drain guard is needed.
            mul = nc.vector.tensor_mul(dst[:, 0:B], a[:, 32:48], tct[:])
            gate_dve[0] = mul
            for k in range(1, KT):
                rdests = [None] * KT
                rdests[k] = (0, k ^ 2 if k >= 4 else k)
                nc.gpsimd.remote_dma_broadcast(
                    dst[:, k * B : (k + 1) * B],
                    dst[:, 0:B],
                    remote_sem=rsems[k - 1],
                    local_sem=lsem,
                    rdests=rdests,
                )
            nc.gpsimd.trigger_dma(count=None)
            return
        if rdma:
            # write h straight into position 0 of this step's hist slot; the
            # recursive-doubling rounds read/extend dst in place, so no
            # separate h buffer and no send-drain guard are needed.
            mul = nc.vector.tensor_mul(dst[:, 0:B], a[:, 32:48], tct[:])
            gate_dve[0] = mul
            # round 1: my slice (pos 0) -> peer^1's pos 1
            r1 = nc.gpsimd.remote_dma_broadcast(
                dst[:, B : 2 * B], dst[:, 0:B],
                remote_sem=rsems[0], local_sem=lsem,
                rdests=[None, (0, 1)] + [None] * 6,
            )
            t1 = nc.gpsimd.trigger_dma(count=None)
            _anchor(t1, r1)
            # round 2: pos 0-1 -> peer^2's pos 2-3 (after my pos 1 arrived)
            r2 = nc.gpsimd.remote_dma_broadcast(
                dst[:, 2 * B : 4 * B], dst[:, 0 : 2 * B],
                remote_sem=rsems[1], local_sem=lsem,
                rdests=[None, None, (0, 2)] + [None] * 5,
            )
            _anchor(r2, t1)
            mv = nc.gpsimd.reg_mov(regs["pool"], 2 * g)
            _anchor(mv, r2)
            w1 = nc.gpsimd.wait_ge(rsems[0], regs["pool"])
            _anchor(w1, mv)
            t2 = nc.gpsimd.trigger_dma(count=None)
            _anchor(t2, w1)
            # round 3: pos 0-3 -> peer^4's pos 4-7 (after my pos 2-3 arrived);
            # logical ^4 is physical ^6 (die crossing flips bit 1).
            r3 = nc.gpsimd.remote_dma_broadcast(
                dst[:, 4 * B : 8 * B], dst[:, 0 : 4 * B],
                remote_sem=rsems[2], local_sem=lsem,
                rdests=[None] * 6 + [(0, 6), None],
            )
            _anchor(r3, t2)
            w2 = nc.gpsimd.wait_ge(rsems[1], regs["pool"])
            _anchor(w2, r3)
            t3 = nc.gpsimd.trigger_dma(count=None)
            _anchor(t3, w2)
            return
        h_sb = hp.tile([128, B], DT.bfloat16, tag="h")
        mul = nc.vector.tensor_mul(h_sb[:], a[:, 32:48], tct[:])
        gate_dve[0] = mul

        # ---- exchange ----
        if CC_MODE == "sbuf":
            hs = h_sb[:]
            nc.sync.dma_start(
                dst.rearrange("p (k b) -> p k b", k=KT),
                bass.AP(hs.tensor, hs.offset, [list(hs.ap[0]), [0, KT], [1, B]]),
            )
            return
        eng = {"gpsimd": nc.gpsimd, "act": nc.scalar}.get(CC_DMA, nc.sync)
        cin = dcc.tile([128, B], DT.bfloat16, tag="cin")
        cout = dccs.tile(
            [NC * 128, B], DT.bfloat16, tag="cout",
            addr_space="Shared" if CC_SHARED else "Local",
        )
        eng.dma_start(cin[:], h_sb[:])
        if CC_MODE == "cc":
            nc.gpsimd.collective_compute(
                "AllGather",
                ALU.bypass,
                replica_groups=[list(range(NC))],
                ins=[cin[:].opt()],
                outs=[cout[:].opt()],
            )
        else:  # "dram"
            ci = cin[:]
            eng.dma_start(
                cout[:].rearrange("(k p) b -> k p b", p=128),
                bass.AP(ci.tensor, ci.offset, [[0, KT], [B, 128], [1, B]]),
            )
        eng.dma_start(
            dst.rearrange("p (k b) -> p k b", k=KT),
            cout[:].rearrange("(k p) b -> p k b", p=128),
        )

    # ================= background tasks =================
    def fc_chunk(mj, nb):
        f1t = ws.tile([128, KT * 128], DT.bfloat16, tag="f1t")
        nc.sync.dma_start(f1t[:], fw1[:, mj * 1024 : (mj + 1) * 1024])
        ps = paux.tile([128, 512], DT.float32, tag="aux")
        hv = hist_d[:].rearrange("p (t k b) -> p t k b", t=n_steps, k=KT)
        thr = 2 * (rep * 2 * n_steps + n_steps + min((nb + 1) * 32, n_steps))
        w = anchored_waits(nc.tensor, regs["pe"], rsems, thr, gate_pe) if rdma else None
        for k in range(KT):
            mm = nc.tensor.matmul(
                ps[:],
                lhsT=f1t[:, k * 128 : (k + 1) * 128],
                rhs=hv[:, nb * 32 : (nb + 1) * 32, k : k + 1, :],
                start=(k == 0),
                stop=(k == KT - 1),
            )
            if k == 0 and w is not None:
                _anchor(mm, w)
        gate_pe[0] = mm

        fco = ws.tile([128, 512], DT.float32, tag="fco")
        b2 = bias2_sb[:, mj * B : (mj + 1) * B]
        nc.vector.tensor_tensor(
            fco[:].rearrange("p (t b) -> p t b", t=32),
            ps[:].rearrange("p (t b) -> p t b", t=32),
            _bcast(b2, 1, 32),
            op=ALU.add,
        )
        nc.sync.dma_start(
            out[mj * 128 : (mj + 1) * 128, nb * 512 : (nb + 1) * 512], fco[:]
        )

    def se_quarter(q):
        ps = paux.tile([1, 512], DT.float32, tag="aux")
        hv = hist_e[:].rearrange("p (t k b) -> p t k b", t=n_steps, k=KT)
        thr = 2 * (rep * 2 * n_steps + n_steps)
        w = anchored_waits(nc.tensor, regs["pe"], rsems, thr, gate_pe) if rdma else None
        for k in range(KT):
            mm = nc.tensor.matmul(
                ps[:],
                lhsT=wet_sb[:, k : k + 1],
                rhs=hv[:, q * 32 : (q + 1) * 32, k : k + 1, :],
                start=(k == 0),
                stop=(k == KT - 1),
            )
            if k == 0 and w is not None:
                _anchor(mm, w)
        gate_pe[0] = mm

        nc.scalar.activation(se_sb[:, q * 512 : (q + 1) * 512], ps[:], AF.Identity)

    se_dram = dcc.tile([1, T2], DT.float32, tag="sed")
    abc_dram = dcc.tile([B, n_steps], DT.bfloat16, tag="abcd")

    def softmax_pieces():
        yield lambda: nc.sync.dma_start(se_dram[:], se_sb[:])
        # se layout: free = e*16 + b; load transposed [b, e]
        yield lambda: nc.sync.dma_start(
            seT_sb[:],
            bass.AP(se_dram[:].tensor, se_dram[:].offset, [[1, B], [B, n_steps]]),
        )

        def red():
            mx = wk.tile([B, 1], DT.float32, tag="mx")
            nc.vector.reduce_max(mx[:], seT_sb[:], axis=mybir.AxisListType.X)
            nmx = wk.tile([B, 1], DT.float32, tag="nmx")
            nc.vector.tensor_scalar(nmx[:], mx[:], -1.0, None, op0=ALU.mult)
            ex = wk1.tile([B, n_steps], DT.float32, tag="ex")
            nc.scalar.activation(ex[:], seT_sb[:], AF.Exp, bias=nmx[:])
            sm = wk.tile([B, 1], DT.float32, tag="sm")
            nc.vector.reduce_sum(sm[:], ex[:], axis=mybir.AxisListType.X)
            rs = wk.tile([B, 1], DT.float32, tag="rs")
            nc.vector.reciprocal(rs[:], sm[:])
            nc.vector.tensor_scalar(attn_sb[:], ex[:], rs[:], None, op0=ALU.mult)

        yield red
        yield lambda: nc.sync.dma_start(abc_dram[:], attn_sb[:])
        yield lambda: nc.sync.dma_start(
            abc_sb[:],
            bass.AP(abc_dram[:].tensor, abc_dram[:].offset, [[0, 128], [1, B * n_steps]]),
        )

    def ctx_chunk(k, bh):
        tmp = wk1.tile([128, 8 * n_steps], DT.float32, tag="ctmp")
        he = hist_e[:].rearrange("p (e k b) -> p k b e", e=n_steps, k=KT)
        ab = abc_sb[:].rearrange("p (b e) -> p b e", b=B)
        b0 = bh * 8
        thr = 2 * (rep * 2 * n_steps + n_steps)
        w = anchored_waits(nc.vector, regs["dve"], rsems, thr, gate_dve) if rdma else None
        tt = nc.vector.tensor_tensor(
            tmp[:].rearrange("p (b e) -> p b e", b=8),
            he[:, k, b0 : b0 + 8, :],
            ab[:, b0 : b0 + 8, :],
            op=ALU.mult,
        )
        if w is not None:
            _anchor(tt, w)
        gate_dve[0] = tt

        ctf = wk.tile([128, 8], DT.float32, tag="ctf")
        nc.vector.reduce_sum(
            ctf[:],
            tmp[:].rearrange("p (b e) -> p b e", b=8),
            axis=mybir.AxisListType.X,
        )
        nc.vector.tensor_copy(ctx_sb[:, k * B + b0 : k * B + b0 + 8], ctf[:])

    def bias2_chunk(mj):
        f2t = ws.tile([128, KT * 128], DT.bfloat16, tag="f2t")
        nc.sync.dma_start(f2t[:], fw2[:, mj * 1024 : (mj + 1) * 1024])
        ps = paux.tile([128, B], DT.float32, tag="aux")
        for k in range(KT):
            nc.tensor.matmul(
                ps[:],
                lhsT=f2t[:, k * 128 : (k + 1) * 128],
                rhs=ctx_sb[:, k * B : (k + 1) * B],
                start=(k == 0),
                stop=(k == KT - 1),
            )
        nc.scalar.activation(
            bias2_sb[:, mj * B : (mj + 1) * B],
            ps[:],
            AF.Identity,
            bias=fcb_sb[:, mj : mj + 1],
        )

    # ================= emission =================
    load_xt(xet)
    for j in range(4):
        for nb in range(4):
            xw_chunk(xwe_sb, wie_sb, be_sb, j, nb)

    enc_bg = [lambda s=src: load_xt(s) for src in (xdt,)]
    enc_bg += [
        (lambda j=j, nb=nb: xw_chunk(xwd_sb, wid_sb, bd_sb, j, nb))
        for j in range(4)
        for nb in range(4)
    ]
    for t in range(n_steps):
        gstep[0] += 1
        hprev = h0_sb[:] if t == 0 else hist_e[:, (t - 1) * 128 : t * 128]
        lstm_step(hist_e, hprev, xwe_sb, whe_sb, t, gstep[0])
        if not NO_BG and t >= 2 and t % 6 == 2 and enc_bg:
            enc_bg.pop(0)()

    dec_bg = [(lambda q=q: se_quarter(q)) for q in range(4)]
    dec_bg += list(softmax_pieces())
    dec_bg += [
        (lambda k=k, bh=bh: ctx_chunk(k, bh)) for k in range(KT) for bh in range(2)
    ]
    dec_bg += [(lambda mj=mj: bias2_chunk(mj)) for mj in range(MT)]
    fc_ready = {nb: 32 * (nb + 1) + 1 for nb in range(4)}
    fc_tasks = [(nb, mj) for nb in range(4) for mj in range(MT)]
    fc_i = 0
    for t in range(n_steps):
        gstep[0] += 1
        hprev = (
            hist_e[:, (n_steps - 1) * 128 : n_steps * 128]
            if t == 0
            else hist_d[:, (t - 1) * 128 : t * 128]
        )
        lstm_step(hist_d, hprev, xwd_sb, whd_sb, t, gstep[0])
        if NO_BG:
            continue
        if t >= 1 and dec_bg:
            dec_bg.pop(0)()
            if dec_bg:
                dec_bg.pop(0)()
        elif fc_i < len(fc_tasks) and t >= fc_ready[fc_tasks[fc_i][0]]:
            nb, mj = fc_tasks[fc_i]
            fc_chunk(mj, nb)
            fc_i += 1
    if NO_BG:
        for e in enc_bg:
            e()
        return
    while fc_i < len(fc_tasks):
        nb, mj = fc_tasks[fc_i]
        fc_chunk(mj, nb)
        fc_i += 1

    if DEBUG_DUMP:
        hw = n_steps * 128
        nch = max(1, hw // 2048)
        cw = hw // nch
        if rdma:
            w = anchored_waits(
                nc.vector, regs["dve"], rsems, 2 * (rep + 1) * 2 * n_steps, gate_dve
            )
            gate_dve[0] = w
        for hi, hsrc in ((0, hist_e), (1024, hist_d)):
            for ch in range(nch):
                dbg = wk1.tile([128, 2048], DT.float32, tag="dbg")
                cp = nc.vector.tensor_copy(dbg[:, 0:cw], hsrc[:, ch * cw : (ch + 1) * cw])
                if rdma:
                    _anchor(cp, gate_dve[0])

                nc.sync.dma_start(out[hi + ch * 128 : hi + (ch + 1) * 128, 0:cw], dbg[:, 0:cw])


# ---------------- host side ----------------


def _gate_rows(m):
    return np.concatenate(
        [np.arange(g * H + m * HL, g * H + (m + 1) * HL) for g in GATE_SRC]
    )


def _kperm(m):
    """K-block permutation for core m: position k holds h-slice (m ^ k)."""
    if CC_MODE == "rdma":
        return [m ^ k for k in range(KT)]
    return list(range(KT))


def _pack_whh(w, rows, m):
    """[4H, H] weight -> per-core [128, KT*512] bf16 sbuf layout (k, j, c)."""
    lhsT = np.ascontiguousarray(w[rows].T)  # [1024, 512]
    blk = lhsT.reshape(KT, 128, 4, 128)[_kperm(m)]
    return blk.transpose(1, 0, 2, 3).reshape(128, KT * 512).astype(BF16)


def _pack_fc(wpart, m):
    """[4096, 1024] -> [128, MT*KT*128] bf16 layout (mj, k, c)."""
    lhsT = np.ascontiguousarray(wpart.T)  # [1024, 4096]
    blk = lhsT.reshape(KT, 128, MT, 128)[_kperm(m)]
    return blk.transpose(1, 2, 0, 3).reshape(128, MT * KT * 128).astype(BF16)


def _xT(emb_rows):
    """[B, T, H] f32 -> [H, T2] bf16 with tau = t*B + b."""
    xt = np.transpose(emb_rows, (1, 0, 2)).reshape(T2, H)
    return np.ascontiguousarray(xt.T).astype(BF16)


_NC_CACHE = {}


def _get_nc():
    if "nc" not in _NC_CACHE:
        _NC_CACHE["nc"] = build_nc()
    return _NC_CACHE["nc"]


def make_in_maps(
    src, tgt, src_emb, tgt_emb, enc_Wih, enc_Whh, enc_bih, enc_bhh,
    dec_Wih, dec_Whh, dec_bih, dec_bhh, attn_w, attn_b, fc_w, fc_b,
):
    src = np.asarray(src)
    tgt = np.asarray(tgt)
    xet = _xT(np.asarray(src_emb, np.float32)[src])
    xdt = _xT(np.asarray(tgt_emb, np.float32)[tgt])
    b_enc = np.asarray(enc_bih, np.float32) + np.asarray(enc_bhh, np.float32)
    b_dec = np.asarray(dec_bih, np.float32) + np.asarray(dec_bhh, np.float32)
    we = np.asarray(attn_w, np.float32)[0, H:]
    fc_w = np.asarray(fc_w, np.float32)
    fc_b = np.asarray(fc_b, np.float32)
    ident = np.eye(128, dtype=BF16)

    in_maps = []
    for m in range(NC):
        rows = _gate_rows(m)
        wet_m = np.ascontiguousarray(
            we.reshape(KT, 128)[_kperm(m)].T
        ).astype(BF16)
        vlo = m * VL
        wrows = np.zeros((VLP, 2 * H), np.float32)
        nreal = min(VLP, V - vlo)
        wrows[:nreal] = fc_w[vlo : vlo + nreal]
        brows = np.zeros((VLP,), np.float32)
        brows[:nreal] = fc_b[vlo : vlo + nreal]
        in_maps.append(
            {
                "xet": xet,
                "xdt": xdt,
                "whe": _pack_whh(np.asarray(enc_Whh, np.float32), rows, m),
                "wie": _pack_whh_noperm(np.asarray(enc_Wih, np.float32), rows),
                "whd": _pack_whh(np.asarray(dec_Whh, np.float32), rows, m),
                "wid": _pack_whh_noperm(np.asarray(dec_Wih, np.float32), rows),
                "be": np.ascontiguousarray(b_enc[rows].reshape(4, 128).T),
                "bd": np.ascontiguousarray(b_dec[rows].reshape(4, 128).T),
                "fw1": _pack_fc(wrows[:, :H], m),
                "fw2": _pack_fc(wrows[:, H:], m),
                "fcb": np.ascontiguousarray(brows.reshape(MT, 128).T),
                "wet": wet_m,
                "idt": ident,
            }
        )
    return in_maps


def _pack_whh_noperm(w, rows):
    lhsT = np.ascontiguousarray(w[rows].T)
    return (
        lhsT.reshape(KT, 128, 4, 128).transpose(1, 0, 2, 3).reshape(128, KT * 512)
    ).astype(BF16)


def kernel(**inputs):
    nc = _get_nc()
    in_maps = make_in_maps(**inputs)
    res = run_bass_kernel_spmd(nc, in_maps, core_ids=list(range(NC)))
    shards = [np.asarray(r["out"], np.float32)[:VL] for r in res.results]
    full = np.concatenate(shards, axis=0)  # [V, T2]
    return np.ascontiguousarray(full.reshape(V, T, B).transpose(2, 1, 0))



# revision 28
# speedup vs baseline: 1.2279x; 1.2279x over previous
"""Trainium2 8-core kernel for the LSTM seq2seq + attention + vocab-projection model.

Strategy:
  - LSTM recurrence: tensor-parallel over the gate dimension. Core m owns
    h-slice [m*128:(m+1)*128) and computes the 4 gate rows for that slice
    (packed on host in order [f, i, o, g], 128 rows each). After each step the
    h-slices are exchanged so every core holds the full h for the next step.
  - Exchange modes (CC_MODE):
      "rdma": direct SBUF->SBUF remote_dma_broadcast to each peer (XOR-relative
              single destinations). Receiver position k holds the slice of peer
              (own_id ^ k); all weight K-blocks are XOR-permuted per core on the
              host so the math stays consistent. Per-peer semaphores gate each
              K-tile group of the next matmul, so the exchange overlaps the
              following step's matmuls. Zero DRAM bounces, no ncfw collective.
      "cc":   AllGather via ncfw collective with DRAM bounce buffers.
      "sbuf"/"dram": timing-attribution fakes (wrong results).
  - The additive attention collapses: softmax over (dec@wd + enc@we + b) along
    the encoder axis is independent of the decoder position, so the context
    vector is per-batch constant. ctx[b] = softmax_e(enc_out[b]@we) @ enc_out[b].
  - Final projection is vocab-sharded: core m computes rows [m*4000,(m+1)*4000)
    (padded to 4096) of  out = dec_out @ fc_w[:, :H].T + (ctx @ fc_w[:, H:].T
    + fc_b).  The fc work is emitted as background chunks interleaved into the
    decoder phase so it runs in PE gaps while steps wait on the exchange.
  - The x @ Wih.T + b term is precomputed for all steps (xw) and folded into
    the gate PSUM with an identity-weight matmul, so the per-step cell update
    is ACT/DVE only: sigmoid/tanh straight from PSUM.
Token index convention: tau = t*16 + b  (time-major, batch inner).
"""

import os
import sys

for _p in ("/opt/trn_rl_repo", "/root/.axon_site/_ro/trn_rl_repo"):
    if os.path.isdir(_p) and _p not in sys.path:
        sys.path.insert(0, _p)

import numpy as np
import ml_dtypes

import concourse.bass as bass
import concourse.bacc as bacc
import concourse.tile as tile
from concourse.tile import add_dep_helper
from concourse import mybir
from concourse.bass_utils import run_bass_kernel_spmd

BF16 = ml_dtypes.bfloat16
DT = mybir.dt
AF = mybir.ActivationFunctionType
ALU = mybir.AluOpType

B = 16
T = 128          # both encoder and decoder length
H = 1024
V = 32000
NC = 8
HL = H // NC     # 128  h-slice per core
KT = H // 128    # 8    K tiles of the hidden dim
T2 = B * T       # 2048 tokens
VL = V // NC     # 4000 real vocab rows per core
VLP = 4096       # padded vocab rows per core
MT = VLP // 128  # 32   vocab M-tiles per core
# gate order on device: [f, i, o, g]; torch rows are [i, f, g, o]
GATE_SRC = (1, 0, 3, 2)

CC_MODE = "cc"  # "cc" | "rdma" | "dram" | "sbuf"
RD_EXCHANGE = False  # rdma flavor: True = 3-round recursive doubling, False = 7-peer one-shot
CC_DMA = "sync"  # engine for exchange bounce DMAs: "sync" | "gpsimd" | "act"
CC_SHARED = False  # Shared addr_space for AllGather outputs
DEBUG_DUMP = False
NO_BG = False
DMA_SCRATCH = 16384
H_BUFS = 3


def _bcast(ap, dim, count):
    """Insert a [step=0, count] broadcast dim at position `dim` of ap.ap."""
    l = [list(d) for d in ap.ap]
    l.insert(dim, [0, count])
    return bass.AP(ap.tensor, ap.offset, l)


def build_nc(n_steps=T, reps=1):
    nc = bacc.Bacc(
        "TRN2", target_bir_lowering=False, debug=False, num_devices=NC,
        dynamic_dma_scratch_size=DMA_SCRATCH,
    )

    # ---- kernel I/O (per-core shards; all pre-laid-out on host) ----
    xet = nc.dram_tensor("xet", [H, T2], DT.bfloat16, kind="ExternalInput")
    xdt = nc.dram_tensor("xdt", [H, T2], DT.bfloat16, kind="ExternalInput")
    whe = nc.dram_tensor("whe", [128, KT * 512], DT.bfloat16, kind="ExternalInput")
    wie = nc.dram_tensor("wie", [128, KT * 512], DT.bfloat16, kind="ExternalInput")
    whd = nc.dram_tensor("whd", [128, KT * 512], DT.bfloat16, kind="ExternalInput")
    wid = nc.dram_tensor("wid", [128, KT * 512], DT.bfloat16, kind="ExternalInput")
    be = nc.dram_tensor("be", [128, 4], DT.float32, kind="ExternalInput")
    bd = nc.dram_tensor("bd", [128, 4], DT.float32, kind="ExternalInput")
    fw1 = nc.dram_tensor("fw1", [128, MT * KT * 128], DT.bfloat16, kind="ExternalInput")
    fw2 = nc.dram_tensor("fw2", [128, MT * KT * 128], DT.bfloat16, kind="ExternalInput")
    fcb = nc.dram_tensor("fcb", [128, MT], DT.float32, kind="ExternalInput")
    wet = nc.dram_tensor("wet", [128, KT], DT.bfloat16, kind="ExternalInput")
    idt = nc.dram_tensor("idt", [128, 128], DT.bfloat16, kind="ExternalInput")
    out = nc.dram_tensor("out", [VLP, T2], DT.float32, kind="ExternalOutput")

    rdma = CC_MODE == "rdma"
    # Recursive-doubling exchange: 3 rounds (peer ^1, ^2, ^4), one sem per
    # round. Round r delivers positions [2^(r-1), 2^r) of the XOR-permuted
    # h layout (position k = slice of peer own^k), +2 per step per sem.
    nsems = 3 if RD_EXCHANGE else KT - 1
    rsems = [nc.alloc_semaphore(f"rsem{k}") for k in range(nsems)] if rdma else None
    lsem = nc.alloc_semaphore("lsem") if rdma else None
    # Register-valued wait thresholds: Tile's single-core scheduling sim
    # cannot model remote sem increments (constant-threshold waits on rsems
    # deadlock it), but it treats register-valued thresholds as satisfiable
    # while the HW wait still enforces the real value at runtime.
    regs = None
    if rdma:
        regs = {
            "pe": nc.alloc_register(mybir.EngineType.PE, "pethr"),
            "dve": nc.alloc_register(mybir.EngineType.DVE, "dvethr"),
            "pool": nc.alloc_register(mybir.EngineType.Pool, "poolthr"),
        }

    with tile.TileContext(nc) as tc:
        with (
            tc.tile_pool(name="persist", bufs=1) as pp,
            tc.tile_pool(name="work", bufs=3) as wk,
            tc.tile_pool(name="wstream", bufs=2) as ws,
            tc.tile_pool(name="work1", bufs=1) as wk1,
            tc.tile_pool(name="hpool", bufs=H_BUFS) as hp,
            tc.tile_pool(name="dcc", bufs=4, space="DRAM") as dcc,
            tc.tile_pool(name="dccs", bufs=4, space="DRAM") as dccs,
            tc.tile_pool(name="paux", bufs=3, space="PSUM") as paux,
            tc.tile_pool(name="pgate", bufs=1, space="PSUM") as pgate,
        ):
            gstep = [0]
            for _rep in range(reps):
                _emit_once(
                    nc, tc, pp, wk, wk1, hp, ws, dcc, dccs, paux, pgate, n_steps, gstep,
                    rsems, lsem, regs, _rep,
                    xet, xdt, whe, wie, whd, wid, be, bd, fw1, fw2, fcb, wet,
                    idt, out,
                )

    nc.compile()
    return nc


def _emit_once(
    nc, tc, pp, wk, wk1, hp, ws, dcc, dccs, paux, pgate, n_steps, gstep, rsems, lsem,
    regs, rep,
    xet, xdt, whe, wie, whd, wid, be, bd, fw1, fw2, fcb, wet, idt, out,
):
    rdma = CC_MODE == "rdma"
    gate_pe = [None]   # latest anchor instruction on PE
    gate_dve = [None]  # latest anchor instruction on DVE

    def _anchor(a, b):
        if b is not None:
            add_dep_helper(a.ins, b.ins, sync=False, reason="stream anchor")

    def anchored_waits(eng, reg, sems, thr, anchor):
        """Reg-threshold waits on all rsems at `thr`, chained into the
        engine's anchor stream (reg_mov first, then one wait per sem)."""
        prev = anchor[0]
        mov = eng.reg_mov(reg, thr)
        _anchor(mov, prev)
        prev = mov
        for s in sems:
            w = eng.wait_ge(s, reg)
            _anchor(w, prev)
            prev = w
        return prev
    # ---- persistent SBUF tensors ----
    whe_sb = pp.tile([128, KT * 512], DT.bfloat16, tag="whe")
    whd_sb = pp.tile([128, KT * 512], DT.bfloat16, tag="whd")
    wie_sb = pp.tile([128, KT * 512], DT.bfloat16, tag="wih")
    wid_sb = pp.tile([128, KT * 512], DT.bfloat16, tag="wih")
    be_sb = pp.tile([128, 4], DT.float32, tag="be")
    bd_sb = pp.tile([128, 4], DT.float32, tag="bd")
    fcb_sb = pp.tile([128, MT], DT.float32, tag="fcb")
    wet_sb = pp.tile([128, KT], DT.bfloat16, tag="wet")
    id_sb = pp.tile([128, 128], DT.bfloat16, tag="ident")
    xt_sb = pp.tile([128, KT * T2], DT.bfloat16, tag="xt")       # 32KB/p
    xwe_sb = pp.tile([128, 4 * T2], DT.bfloat16, tag="xwe")      # 16KB/p
    xwd_sb = pp.tile([128, 4 * T2], DT.bfloat16, tag="xwd")      # 16KB/p
    hist_e = pp.tile([128, n_steps * 128], DT.bfloat16, tag="he")
    hist_d = pp.tile([128, n_steps * 128], DT.bfloat16, tag="hd")
    h0_sb = pp.tile([128, 128], DT.bfloat16, tag="h0")
    c_sb = pp.tile([128, B], DT.float32, tag="c")
    se_sb = pp.tile([1, T2], DT.float32, tag="se")
    seT_sb = pp.tile([B, n_steps], DT.float32, tag="seT")
    attn_sb = pp.tile([B, n_steps], DT.bfloat16, tag="attn")
    abc_sb = pp.tile([128, B * n_steps], DT.bfloat16, tag="abc")
    ctx_sb = pp.tile([128, KT * B], DT.bfloat16, tag="ctx")
    bias2_sb = pp.tile([128, MT * B], DT.float32, tag="bias2")

    nc.sync.dma_start(whe_sb[:], whe[:])
    nc.sync.dma_start(whd_sb[:], whd[:])
    nc.sync.dma_start(wie_sb[:], wie[:])
    nc.sync.dma_start(wid_sb[:], wid[:])
    nc.sync.dma_start(be_sb[:], be[:])
    nc.sync.dma_start(bd_sb[:], bd[:])
    nc.sync.dma_start(fcb_sb[:], fcb[:])
    nc.sync.dma_start(wet_sb[:], wet[:])
    nc.sync.dma_start(id_sb[:], idt[:])
    nc.vector.memset(c_sb[:], 0.0)

    # h0 := zeros, doubling as a cross-core barrier in rdma mode: semaphores
    # are cleared before this core contributes to the AllGather, and every
    # core's step-0 matmuls read h0 (the gather output), so no core can send
    # before every core has cleared.
    zin = dcc.tile([128, B], DT.bfloat16, tag="zin")
    zout = dcc.tile([NC * 128, B], DT.bfloat16, tag="zout")
    zsb = wk.tile([128, B], DT.bfloat16, tag="zsb")
    if rdma and rep == 0:
        with tc.tile_critical():
            for s in rsems:
                nc.gpsimd.sem_clear(s)
            nc.gpsimd.sem_clear(lsem)
            nc.gpsimd.memset(zsb[:], 0.0)
    else:
        nc.gpsimd.memset(zsb[:], 0.0)
    nc.gpsimd.dma_start(zin[:], zsb[:])
    nc.gpsimd.collective_compute(
        "AllGather",
        ALU.bypass,
        replica_groups=[list(range(NC))],
        ins=[zin[:].opt()],
        outs=[zout[:].opt()],
    )
    nc.sync.dma_start(
        h0_sb[:].rearrange("p (k b) -> p k b", k=KT),
        zout[:].rearrange("(k p) b -> p k b", p=128),
    )

    def load_xt(src):
        nc.sync.dma_start(
            xt_sb[:].rearrange("p (k n) -> p k n", k=KT),
            src[:].rearrange("(k p) n -> p k n", p=128),
        )

    def xw_chunk(xw_sb, wih_sb, b_sb, j, nb):
        """One (gate j, 512-token block nb) chunk of xw = x @ Wih_m.T + b."""
        ps = paux.tile([128, 512], DT.float32, tag="aux")
        for k in range(KT):
            nc.tensor.matmul(
                ps[:],
                lhsT=wih_sb[:, k * 512 + j * 128 : k * 512 + (j + 1) * 128],
                rhs=xt_sb[:, k * T2 + nb * 512 : k * T2 + (nb + 1) * 512],
                start=(k == 0),
                stop=(k == KT - 1),
            )
        nc.vector.tensor_scalar_add(
            xw_sb[:, j * T2 + nb * 512 : j * T2 + (nb + 1) * 512],
            ps[:],
            b_sb[:, j : j + 1],
        )

    def lstm_step(hist, hprev_ap, xw_sb, whh_sb, t, g):
        """g = global step counter (1-based) for semaphore thresholds."""
        pg = pgate.tile([128, 2048], DT.float32, tag="g")
        # fold xw into PSUM: pg[j] = I.T @ xw_t[j]  (start=True clears)
        for j in range(4):
            nc.tensor.matmul(
                pg[:, j * 512 : j * 512 + B],
                lhsT=id_sb[:],
                rhs=xw_sb[:, j * T2 + t * B : j * T2 + (t + 1) * B],
                start=True,
                stop=False,
            )

        def kmm2(k, last):
            first = None
            for j in range(4):
                mm = nc.tensor.matmul(
                    pg[:, j * 512 : j * 512 + B],
                    lhsT=whh_sb[:, k * 512 + j * 128 : k * 512 + (j + 1) * 128],
                    rhs=hprev_ap[:, k * B : (k + 1) * B],
                    start=False,
                    stop=last,
                )
                if first is None:
                    first = mm
            return first, mm

        def kmm(k, last):
            return kmm2(k, last)[1]

        if rdma and g > 1:
            prev = kmm(0, False)
            mov = nc.tensor.reg_mov(regs["pe"], 2 * (g - 1))
            _anchor(mov, prev)
            prev = mov
            # position k arrives with this sem (RD: per round; one-shot: per peer)
            ksem = {1: 0, 2: 1, 4: 2} if RD_EXCHANGE else {k: k - 1 for k in range(1, KT)}
            for k in range(1, KT):
                si = ksem.get(k)
                if si is not None:
                    w = nc.tensor.wait_ge(rsems[si], regs["pe"])
                    _anchor(w, prev)
                    first, prev = kmm2(k, k == KT - 1)
                    _anchor(first, w)
                else:
                    prev = kmm(k, k == KT - 1)
            gate_pe[0] = prev
        else:
            for k in range(KT):
                last = kmm(k, k == KT - 1)
            gate_pe[0] = last

        # cell update straight from PSUM
        a = wk.tile([128, 64], DT.float32, tag="act")
        nc.scalar.activation(
            a[:].rearrange("p (j b) -> p j b", j=4)[:, 0:3, :],
            pg[:].rearrange("p (j x) -> p j x", j=4)[:, 0:3, 0:B],
            AF.Sigmoid,
        )
        nc.scalar.activation(a[:, 48:64], pg[:, 3 * 512 : 3 * 512 + B], AF.Tanh)
        m1 = wk.tile([128, B], DT.float32, tag="m1")
        m2 = wk.tile([128, B], DT.float32, tag="m2")
        nc.vector.tensor_mul(m1[:], a[:, 0:16], c_sb[:])
        nc.vector.tensor_mul(m2[:], a[:, 16:32], a[:, 48:64])
        nc.vector.tensor_add(c_sb[:], m1[:], m2[:])
        tct = wk.tile([128, B], DT.float32, tag="tct")
        nc.scalar.activation(tct[:], c_sb[:], AF.Tanh)
        dst = hist[:, t * 128 : (t + 1) * 128]
        if rdma and not RD_EXCHANGE:
            # h lands straight in this step's hist slot (position 0); the
            # sends read it from there, so the slot is never reused within a
            # rep and no send-drain guard is needed.
            mul = nc.vector.tensor_mul(dst[:, 0:B], a[:, 32:48], tct[:])
            gate_dve[0] = mul
            for k in range(1, KT):
                rdests = [None] * KT
                rdests[k] = (0, k ^ 2 if k >= 4 else k)
                nc.gpsimd.remote_dma_broadcast(
                    dst[:, k * B : (k + 1) * B],
                    dst[:, 0:B],
                    remote_sem=rsems[k - 1],
                    local_sem=lsem,
                    rdests=rdests,
                )
            nc.gpsimd.trigger_dma(count=None)
            return
        if rdma:
            # write h straight into position 0 of this step's hist slot; the
            # recursive-doubling rounds read/extend dst in place, so no
            # separate h buffer and no send-drain guard are needed.
            mul = nc.vector.tensor_mul(dst[:, 0:B], a[:, 32:48], tct[:])
            gate_dve[0] = mul
            # round 1: my slice (pos 0) -> peer^1's pos 1
            r1 = nc.gpsimd.remote_dma_broadcast(
                dst[:, B : 2 * B], dst[:, 0:B],
                remote_sem=rsems[0], local_sem=lsem,
                rdests=[None, (0, 1)] + [None] * 6,
            )
            t1 = nc.gpsimd.trigger_dma(count=None)
            _anchor(t1, r1)
            # round 2: pos 0-1 -> peer^2's pos 2-3 (after my pos 1 arrived)
            r2 = nc.gpsimd.remote_dma_broadcast(
                dst[:, 2 * B : 4 * B], dst[:, 0 : 2 * B],
                remote_sem=rsems[1], local_sem=lsem,
                rdests=[None, None, (0, 2)] + [None] * 5,
            )
            _anchor(r2, t1)
            mv = nc.gpsimd.reg_mov(regs["pool"], 2 * g)
            _anchor(mv, r2)
            w1 = nc.gpsimd.wait_ge(rsems[0], regs["pool"])
            _anchor(w1, mv)
            t2 = nc.gpsimd.trigger_dma(count=None)
            _anchor(t2, w1)
            # round 3: pos 0-3 -> peer^4's pos 4-7 (after my pos 2-3 arrived);
            # logical ^4 is physical ^6 (die crossing flips bit 1).
            r3 = nc.gpsimd.remote_dma_broadcast(
                dst[:, 4 * B : 8 * B], dst[:, 0 : 4 * B],
                remote_sem=rsems[2], local_sem=lsem,
                rdests=[None] * 6 + [(0, 6), None],
            )
            _anchor(r3, t2)
            w2 = nc.gpsimd.wait_ge(rsems[1], regs["pool"])
            _anchor(w2, r3)
            t3 = nc.gpsimd.trigger_dma(count=None)
            _anchor(t3, w2)
            return
        h_sb = hp.tile([128, B], DT.bfloat16, tag="h")
        mul = nc.vector.tensor_mul(h_sb[:], a[:, 32:48], tct[:])
        gate_dve[0] = mul

        # ---- exchange ----
        if CC_MODE == "sbuf":
            hs = h_sb[:]
            nc.sync.dma_start(
                dst.rearrange("p (k b) -> p k b", k=KT),
                bass.AP(hs.tensor, hs.offset, [list(hs.ap[0]), [0, KT], [1, B]]),
            )
            return
        eng = {"gpsimd": nc.gpsimd, "act": nc.scalar}.get(CC_DMA, nc.sync)
        cin = dcc.tile([128, B], DT.bfloat16, tag="cin")
        cout = dccs.tile(
            [NC * 128, B], DT.bfloat16, tag="cout",
            addr_space="Shared" if CC_SHARED else "Local",
        )
        eng.dma_start(cin[:], h_sb[:])
        if CC_MODE == "cc":
            nc.gpsimd.collective_compute(
                "AllGather",
                ALU.bypass,
                replica_groups=[list(range(NC))],
                ins=[cin[:].opt()],
                outs=[cout[:].opt()],
            )
        else:  # "dram"
            ci = cin[:]
            eng.dma_start(
                cout[:].rearrange("(k p) b -> k p b", p=128),
                bass.AP(ci.tensor, ci.offset, [[0, KT], [B, 128], [1, B]]),
            )
        eng.dma_start(
            dst.rearrange("p (k b) -> p k b", k=KT),
            cout[:].rearrange("(k p) b -> p k b", p=128),
        )

    # ================= background tasks =================
    def fc_chunk(mj, nb):
        f1t = ws.tile([128, KT * 128], DT.bfloat16, tag="f1t")
        nc.sync.dma_start(f1t[:], fw1[:, mj * 1024 : (mj + 1) * 1024])
        ps = paux.tile([128, 512], DT.float32, tag="aux")
        hv = hist_d[:].rearrange("p (t k b) -> p t k b", t=n_steps, k=KT)
        thr = 2 * (rep * 2 * n_steps + n_steps + min((nb + 1) * 32, n_steps))
        w = anchored_waits(nc.tensor, regs["pe"], rsems, thr, gate_pe) if rdma else None
        for k in range(KT):
            mm = nc.tensor.matmul(
                ps[:],
                lhsT=f1t[:, k * 128 : (k + 1) * 128],
                rhs=hv[:, nb * 32 : (nb + 1) * 32, k : k + 1, :],
                start=(k == 0),
                stop=(k == KT - 1),
            )
            if k == 0 and w is not None:
                _anchor(mm, w)
        gate_pe[0] = mm

        fco = ws.tile([128, 512], DT.float32, tag="fco")
        b2 = bias2_sb[:, mj * B : (mj + 1) * B]
        nc.vector.tensor_tensor(
            fco[:].rearrange("p (t b) -> p t b", t=32),
            ps[:].rearrange("p (t b) -> p t b", t=32),
            _bcast(b2, 1, 32),
            op=ALU.add,
        )
        nc.sync.dma_start(
            out[mj * 128 : (mj + 1) * 128, nb * 512 : (nb + 1) * 512], fco[:]
        )

    def se_quarter(q):
        ps = paux.tile([1, 512], DT.float32, tag="aux")
        hv = hist_e[:].rearrange("p (t k b) -> p t k b", t=n_steps, k=KT)
        thr = 2 * (rep * 2 * n_steps + n_steps)
        w = anchored_waits(nc.tensor, regs["pe"], rsems, thr, gate_pe) if rdma else None
        for k in range(KT):
            mm = nc.tensor.matmul(
                ps[:],
                lhsT=wet_sb[:, k : k + 1],
                rhs=hv[:, q * 32 : (q + 1) * 32, k : k + 1, :],
                start=(k == 0),
                stop=(k == KT - 1),
            )
            if k == 0 and w is not None:
                _anchor(mm, w)
        gate_pe[0] = mm

        nc.scalar.activation(se_sb[:, q * 512 : (q + 1) * 512], ps[:], AF.Identity)

    se_dram = dcc.tile([1, T2], DT.float32, tag="sed")
    abc_dram = dcc.tile([B, n_steps], DT.bfloat16, tag="abcd")

    def softmax_pieces():
        yield lambda: nc.sync.dma_start(se_dram[:], se_sb[:])
        # se layout: free = e*16 + b; load transposed [b, e]
        yield lambda: nc.sync.dma_start(
            seT_sb[:],
            bass.AP(se_dram[:].tensor, se_dram[:].offset, [[1, B], [B, n_steps]]),
        )

        def red():
            mx = wk.tile([B, 1], DT.float32, tag="mx")
            nc.vector.reduce_max(mx[:], seT_sb[:], axis=mybir.AxisListType.X)
            nmx = wk.tile([B, 1], DT.float32, tag="nmx")
            nc.vector.tensor_scalar(nmx[:], mx[:], -1.0, None, op0=ALU.mult)
            ex = wk1.tile([B, n_steps], DT.float32, tag="ex")
            nc.scalar.activation(ex[:], seT_sb[:], AF.Exp, bias=nmx[:])
            sm = wk.tile([B, 1], DT.float32, tag="sm")
            nc.vector.reduce_sum(sm[:], ex[:], axis=mybir.AxisListType.X)
            rs = wk.tile([B, 1], DT.float32, tag="rs")
            nc.vector.reciprocal(rs[:], sm[:])
            nc.vector.tensor_scalar(attn_sb[:], ex[:], rs[:], None, op0=ALU.mult)

        yield red
        yield lambda: nc.sync.dma_start(abc_dram[:], attn_sb[:])
        yield lambda: nc.sync.dma_start(
            abc_sb[:],
            bass.AP(abc_dram[:].tensor, abc_dram[:].offset, [[0, 128], [1, B * n_steps]]),
        )

    def ctx_chunk(k, bh):
        tmp = wk1.tile([128, 8 * n_steps], DT.float32, tag="ctmp")
        he = hist_e[:].rearrange("p (e k b) -> p k b e", e=n_steps, k=KT)
        ab = abc_sb[:].rearrange("p (b e) -> p b e", b=B)
        b0 = bh * 8
        thr = 2 * (rep * 2 * n_steps + n_steps)
        w = anchored_waits(nc.vector, regs["dve"], rsems, thr, gate_dve) if rdma else None
        tt = nc.vector.tensor_tensor(
            tmp[:].rearrange("p (b e) -> p b e", b=8),
            he[:, k, b0 : b0 + 8, :],
            ab[:, b0 : b0 + 8, :],
            op=ALU.mult,
        )
        if w is not None:
            _anchor(tt, w)
        gate_dve[0] = tt

        ctf = wk.tile([128, 8], DT.float32, tag="ctf")
        nc.vector.reduce_sum(
            ctf[:],
            tmp[:].rearrange("p (b e) -> p b e", b=8),
            axis=mybir.AxisListType.X,
        )
        nc.vector.tensor_copy(ctx_sb[:, k * B + b0 : k * B + b0 + 8], ctf[:])

    def bias2_chunk(mj):
        f2t = ws.tile([128, KT * 128], DT.bfloat16, tag="f2t")
        nc.sync.dma_start(f2t[:], fw2[:, mj * 1024 : (mj + 1) * 1024])
        ps = paux.tile([128, B], DT.float32, tag="aux")
        for k in range(KT):
            nc.tensor.matmul(
                ps[:],
                lhsT=f2t[:, k * 128 : (k + 1) * 128],
                rhs=ctx_sb[:, k * B : (k + 1) * B],
                start=(k == 0),
                stop=(k == KT - 1),
            )
        nc.scalar.activation(
            bias2_sb[:, mj * B : (mj + 1) * B],
            ps[:],
            AF.Identity,
            bias=fcb_sb[:, mj : mj + 1],
        )

    # ================= emission =================
    load_xt(xet)
    for j in range(4):
        for nb in range(4):
            xw_chunk(xwe_sb, wie_sb, be_sb, j, nb)

    enc_bg = [lambda s=src: load_xt(s) for src in (xdt,)]
    enc_bg += [
        (lambda j=j, nb=nb: xw_chunk(xwd_sb, wid_sb, bd_sb, j, nb))
        for j in range(4)
        for nb in range(4)
    ]
    for t in range(n_steps):
        gstep[0] += 1
        hprev = h0_sb[:] if t == 0 else hist_e[:, (t - 1) * 128 : t * 128]
        lstm_step(hist_e, hprev, xwe_sb, whe_sb, t, gstep[0])
        if not NO_BG and t >= 2 and t % 6 == 2 and enc_bg:
            enc_bg.pop(0)()

    dec_bg = [(lambda q=q: se_quarter(q)) for q in range(4)]
    dec_bg += list(softmax_pieces())
    dec_bg += [
        (lambda k=k, bh=bh: ctx_chunk(k, bh)) for k in range(KT) for bh in range(2)
    ]
    dec_bg += [(lambda mj=mj: bias2_chunk(mj)) for mj in range(MT)]
    fc_ready = {nb: 32 * (nb + 1) + 1 for nb in range(4)}
    fc_tasks = [(nb, mj) for nb in range(4) for mj in range(MT)]
    fc_i = 0
    for t in range(n_steps):
        gstep[0] += 1
        hprev = (
            hist_e[:, (n_steps - 1) * 128 : n_steps * 128]
            if t == 0
            else hist_d[:, (t - 1) * 128 : t * 128]
        )
        lstm_step(hist_d, hprev, xwd_sb, whd_sb, t, gstep[0])
        if NO_BG:
            continue
        if t >= 1 and dec_bg:
            dec_bg.pop(0)()
            if dec_bg:
                dec_bg.pop(0)()
        elif fc_i < len(fc_tasks) and t >= fc_ready[fc_tasks[fc_i][0]]:
            nb, mj = fc_tasks[fc_i]
            fc_chunk(mj, nb)
            fc_i += 1
    if NO_BG:
        for e in enc_bg:
            e()
        return
    while fc_i < len(fc_tasks):
        nb, mj = fc_tasks[fc_i]
        fc_chunk(mj, nb)
        fc_i += 1

    if DEBUG_DUMP:
        hw = n_steps * 128
        nch = max(1, hw // 2048)
        cw = hw // nch
        if rdma:
            w = anchored_waits(
                nc.vector, regs["dve"], rsems, 2 * (rep + 1) * 2 * n_steps, gate_dve
            )
            gate_dve[0] = w
        for hi, hsrc in ((0, hist_e), (1024, hist_d)):
            for ch in range(nch):
                dbg = wk1.tile([128, 2048], DT.float32, tag="dbg")
                cp = nc.vector.tensor_copy(dbg[:, 0:cw], hsrc[:, ch * cw : (ch + 1) * cw])
                if rdma:
                    _anchor(cp, gate_dve[0])

                nc.sync.dma_start(out[hi + ch * 128 : hi + (ch + 1) * 128, 0:cw], dbg[:, 0:cw])


# ---------------- host side ----------------


def _gate_rows(m):
    return np.concatenate(
        [np.arange(g * H + m * HL, g * H + (m + 1) * HL) for g in GATE_SRC]
    )


def _kperm(m):
    """K-block permutation for core m: position k holds h-slice (m ^ k)."""
    if CC_MODE == "rdma":
        return [m ^ k for k in range(KT)]
    return list(range(KT))


def _pack_whh(w, rows, m):
    """[4H, H] weight -> per-core [128, KT*512] bf16 sbuf layout (k, j, c)."""
    lhsT = np.ascontiguousarray(w[rows].T)  # [1024, 512]
    blk = lhsT.reshape(KT, 128, 4, 128)[_kperm(m)]
    return blk.transpose(1, 0, 2, 3).reshape(128, KT * 512).astype(BF16)


def _pack_fc(wpart, m):
    """[4096, 1024] -> [128, MT*KT*128] bf16 layout (mj, k, c)."""
    lhsT = np.ascontiguousarray(wpart.T)  # [1024, 4096]
    blk = lhsT.reshape(KT, 128, MT, 128)[_kperm(m)]
    return blk.transpose(1, 2, 0, 3).reshape(128, MT * KT * 128).astype(BF16)


def _xT(emb_rows):
    """[B, T, H] f32 -> [H, T2] bf16 with tau = t*B + b."""
    xt = np.transpose(emb_rows, (1, 0, 2)).reshape(T2, H)
    return np.ascontiguousarray(xt.T).astype(BF16)


_NC_CACHE = {}


def _get_nc():
    if "nc" not in _NC_CACHE:
        _NC_CACHE["nc"] = build_nc()
    return _NC_CACHE["nc"]


def make_in_maps(
    src, tgt, src_emb, tgt_emb, enc_Wih, enc_Whh, enc_bih, enc_bhh,
    dec_Wih, dec_Whh, dec_bih, dec_bhh, attn_w, attn_b, fc_w, fc_b,
):
    src = np.asarray(src)
    tgt = np.asarray(tgt)
    xet = _xT(np.asarray(src_emb, np.float32)[src])
    xdt = _xT(np.asarray(tgt_emb, np.float32)[tgt])
    b_enc = np.asarray(enc_bih, np.float32) + np.asarray(enc_bhh, np.float32)
    b_dec = np.asarray(dec_bih, np.float32) + np.asarray(dec_bhh, np.float32)
    we = np.asarray(attn_w, np.float32)[0, H:]
    fc_w = np.asarray(fc_w, np.float32)
    fc_b = np.asarray(fc_b, np.float32)
    ident = np.eye(128, dtype=BF16)

    in_maps = []
    for m in range(NC):
        rows = _gate_rows(m)
        wet_m = np.ascontiguousarray(
            we.reshape(KT, 128)[_kperm(m)].T
        ).astype(BF16)
        vlo = m * VL
        wrows = np.zeros((VLP, 2 * H), np.float32)
        nreal = min(VLP, V - vlo)
        wrows[:nreal] = fc_w[vlo : vlo + nreal]
        brows = np.zeros((VLP,), np.float32)
        brows[:nreal] = fc_b[vlo : vlo + nreal]
        in_maps.append(
            {
                "xet": xet,
                "xdt": xdt,
                "whe": _pack_whh(np.asarray(enc_Whh, np.float32), rows, m),
                "wie": _pack_whh_noperm(np.asarray(enc_Wih, np.float32), rows),
                "whd": _pack_whh(np.asarray(dec_Whh, np.float32), rows, m),
                "wid": _pack_whh_noperm(np.asarray(dec_Wih, np.float32), rows),
                "be": np.ascontiguousarray(b_enc[rows].reshape(4, 128).T),
                "bd": np.ascontiguousarray(b_dec[rows].reshape(4, 128).T),
                "fw1": _pack_fc(wrows[:, :H], m),
                "fw2": _pack_fc(wrows[:, H:], m),
                "fcb": np.ascontiguousarray(brows.reshape(MT, 128).T),
                "wet": wet_m,
                "idt": ident,
            }
        )
    return in_maps


def _pack_whh_noperm(w, rows):
    lhsT = np.ascontiguousarray(w[rows].T)
    return (
        lhsT.reshape(KT, 128, 4, 128).transpose(1, 0, 2, 3).reshape(128, KT * 512)
    ).astype(BF16)


def kernel(**inputs):
    nc = _get_nc()
    in_maps = make_in_maps(**inputs)
    res = run_bass_kernel_spmd(nc, in_maps, core_ids=list(range(NC)))
    shards = [np.asarray(r["out"], np.float32)[:VL] for r in res.results]
    full = np.concatenate(shards, axis=0)  # [V, T2]
    return np.ascontiguousarray(full.reshape(V, T, B).transpose(2, 1, 0))

